# revision 1
# baseline (speedup 1.0000x reference)
"""Trainium2 Bass kernel for additive-attention energies + softmax.

Computes, for hidden [1, B, H], encoder_outputs [T, B, H], W [H, H], b [H]:
    proj[t,b,o]  = sum_h enc[t,b,h] * W[o,h] + b[o]
    energies[b,t] = sum_o hidden[0,b,o] * proj[t,b,o]
    out = softmax(energies, axis=-1)[:, None, :]            # [B, 1, T]

Algebraic rewrite used on-device:
    energies[b,t] = (hidden[b] @ W) . enc[t,b]  +  hidden[b] . b
The second term is constant in t, so it drops out of the softmax entirely.
v = hidden @ W is a tiny [B, H] matmul done on the tensor engine (fp32,
column-tiled so both h-halves run concurrently in the 128x128 array); v is
then broadcast across partitions with indicator-matrix matmuls. The
dominant work is streaming the 256 MB of encoder outputs once and a fused
multiply+reduce per (t-chunk, b) on the vector engine
(scalar_tensor_tensor with accum_out). Energies are transposed back via
PE-transpose; softmax runs on [8, 1024] rows at the end.

Sharding: data-parallel over batch. Core i handles batches [8i, 8i+8):
  enc slice [T, 8, H] (32 MB), hidden-transpose slice [H, 8], W replicated.
Per-core output is [8, T]; host concatenates to [B, 1, T].
No cross-core communication. Per-core roofline: ~36 MB of HBM reads at
~360 GB/s ~= 100 us; measured ~121-125 us end-to-end (incl. ~8.5 us NEFF
preamble and kernel tail).
"""

import sys

import numpy as np

for _p in ("/opt/trn_rl_repo",):
    if _p not in sys.path:
        sys.path.insert(0, _p)

T, B, H = 1024, 64, 1024
NCORES = 8
BPC = B // NCORES  # batches per core
TCH = 128          # t-chunk = SBUF partition count
NTCH = T // TCH
ENC_BUFS = 3

_BASS_CACHE = {}


def _split_multi_waits(nc):
    """This walrus build rejects >1 semaphore wait per instruction for
    several instruction types (Drain/CTRL, LDWEIGHTS, ...). Normalize every
    instruction to <=1 wait: hoist extra waits onto fresh single-wait drain
    clones inserted immediately before it on the same engine (engines are
    serial, so semantics are identical)."""
    import copy

    template = None
    for fn in nc.m.functions:
        for bb in fn.blocks:
            for inst in bb.instructions:
                if type(inst).__name__ == "InstDrain":
                    template = inst
                    break
            if template is not None:
                break
        if template is not None:
            break
    assert template is not None, "no InstDrain found to use as wait-carrier"

    uid = [0]
    for fn in nc.m.functions:
        for bb in fn.blocks:
            out = []
            changed = False
            for inst in bb.instructions:
                si = inst.sync_info
                if si is not None and si.on_wait and len(si.on_wait) > 1:
                    waits = list(si.on_wait)
                    for w in waits[:-1]:
                        d = copy.deepcopy(template)
                        d.name = f"waitsplit-{uid[0]}"
                        uid[0] += 1
                        d.engine = inst.engine
                        dsi = d.sync_info
                        dsi.on_wait = [w]
                        if dsi.on_update:
                            dsi.on_update = []
                        out.append(d)
                        nc.register_instruction(d, overwrite=True)
                    si.on_wait = [waits[-1]]
                    changed = True
                out.append(inst)
            if changed:
                try:
                    bb.instructions = out
                except Exception:
                    bb.instructions.clear()
                    bb.instructions.extend(out)


def _build_bass():
    """Build the per-core Bass program (same program on all 8 cores)."""
    from contextlib import ExitStack

    import concourse.bass as bass
    import concourse.mybir as mybir
    import concourse.tile as tile
    from concourse.masks import make_identity

    f32 = mybir.dt.float32
    Alu = mybir.AluOpType

    nc = bass.Bass("TRN2")
    enc_h = nc.dram_tensor("enc", [T, BPC, H], f32, kind="ExternalInput")
    # hidt arrives host-prearranged as [128, H/128 * BPC] = the exact SBUF
    # tile layout, so its DMA is one contiguous 256B run per partition
    hid_h = nc.dram_tensor("hidt", [128, (H // 128) * BPC], f32, kind="ExternalInput")
    w_h = nc.dram_tensor("w", [H, H], f32, kind="ExternalInput")
    out_h = nc.dram_tensor("out", [BPC, T], f32, kind="ExternalOutput")

    enc, hidT, w, out = enc_h.ap(), hid_h.ap(), w_h.ap(), out_h.ap()

    with tile.TileContext(nc) as tc, ExitStack() as ctx:
        const = ctx.enter_context(tc.tile_pool(name="const", bufs=1))
        wpool = ctx.enter_context(tc.tile_pool(name="wpool", bufs=1))
        vpool = ctx.enter_context(tc.tile_pool(name="vpool", bufs=1))
        vbpool = ctx.enter_context(tc.tile_pool(name="vb", bufs=1))
        encpool = ctx.enter_context(tc.tile_pool(name="encp", bufs=ENC_BUFS))
        encq = ctx.enter_context(tc.tile_pool(name="encq", bufs=1))
        scrpool = ctx.enter_context(tc.tile_pool(name="scr", bufs=2))
        epool = ctx.enter_context(tc.tile_pool(name="ep", bufs=1))
        smpool = ctx.enter_context(tc.tile_pool(name="sm", bufs=1))
        psv = ctx.enter_context(tc.tile_pool(name="psv", bufs=1, space="PSUM"))
        psw = ctx.enter_context(tc.tile_pool(name="psw", bufs=1, space="PSUM"))
        psb = ctx.enter_context(tc.tile_pool(name="psb", bufs=4, space="PSUM"))
        pse = ctx.enter_context(tc.tile_pool(name="pse", bufs=1, space="PSUM"))

        # ind[k, b*128 + m] = 1 if k == b else 0; used as matmul lhsT to
        # broadcast row b of a [BPC, N] SBUF tile across 128 partitions.
        # Constants built on-device (gpsimd) so no DMA gates the PE warm-up.
        # ident: 128x128 identity for PE transposes.
        ident = const.tile([128, 128], f32)
        make_identity(nc, ident[:])
        # ind2[k, b*128 + m] = 1 if k == b (b = f//128) else 0. Used as a
        # K=128 matmul lhsT that broadcasts row b of v_pad across all 128
        # output partitions while zero-killing the 120 garbage pad rows.
        ind2 = const.tile([128, BPC * 128], f32)
        nc.gpsimd.memset(ind2[:], 1.0)
        nc.gpsimd.affine_select(
            out=ind2[:], in_=ind2[:], compare_op=Alu.is_ge, fill=0.0,
            base=0, pattern=[[1, BPC * 128]], channel_multiplier=-128,
        )
        nc.gpsimd.affine_select(
            out=ind2[:], in_=ind2[:], compare_op=Alu.is_ge, fill=0.0,
            base=127, pattern=[[-1, BPC * 128]], channel_multiplier=128,
        )
        # ind3: same but selecting k == b + 32 (for the col-tiled v half
        # whose PSUM lives on partitions 32..32+BPC)
        ind3 = const.tile([128, BPC * 128], f32)
        nc.gpsimd.memset(ind3[:], 1.0)
        nc.gpsimd.affine_select(
            out=ind3[:], in_=ind3[:], compare_op=Alu.is_ge, fill=0.0,
            base=32 * 128, pattern=[[1, BPC * 128]], channel_multiplier=-128,
        )
        nc.gpsimd.affine_select(
            out=ind3[:], in_=ind3[:], compare_op=Alu.is_ge, fill=0.0,
            base=127 - 32 * 128, pattern=[[-1, BPC * 128]], channel_multiplier=128,
        )

        # Preload the ScalarE activation table (Copy lives in the same set
        # as Exp) during the preamble -- otherwise the first ACT copy on the
        # v->broadcast critical path eats a ~2.7us ACT_TABLE_LOAD.
        actwarm = const.tile([1, 1], f32)
        nc.scalar.activation(actwarm[:], actwarm[:],
                             mybir.ActivationFunctionType.Exp)

        # PE warm-up: junk matmuls so the HAM un-throttles the PE clock
        # (1.2 -> 2.4 GHz) before the v/broadcast matmul chain, which is on
        # the critical path to the vector engine's first stream op. Kept
        # short enough not to block the first chunk-paced v matmuls.
        for wi in range(8):
            pw = psw.tile([128, 128], f32, tag="warm")
            nc.tensor.matmul(pw[:], lhsT=ident[:], rhs=ident[:], start=True, stop=True)

        # hidT (prearranged) -> SBUF [128, H/128, BPC] (o on partitions)
        hid_sb = const.tile([128, H // 128, BPC], f32)
        nc.sync.dma_start(hid_sb[:], hidT.rearrange("p (oc b) -> p oc b", b=BPC))

        # W [o, h] -> SBUF [128, 8, H], one DMA per 512KB o-chunk so the
        # v matmuls can start as soon as their chunk lands.
        w_sb = wpool.tile([128, H // 128, H], f32)
        w_r = w.rearrange("(oc p) h -> p oc h", p=128)
        for oc in range(H // 128):
            nc.sync.dma_start(w_sb[:, oc, :], w_r[:, oc, :])

        # v[b, h] = sum_o hidden[b, o] W[o, h] -> [BPC, H] via PE. The two
        # h-halves interleave per o-chunk so matmuls trail the W chunk DMAs.
        # v_pad is [128, H] with rows BPC..127 zeroed, so the K=128
        # broadcast matmul below can contract over all 128 partitions.
        v_pad = vpool.tile([128, H], f32)
        nc.vector.memset(v_pad[:], 0.0)
        # col-tiled: half 0 in array cols 0-31 -> PSUM partitions 0..BPC,
        # half 1 in array cols 32-63 -> PSUM partitions 32..32+BPC; the two
        # halves' matmuls run concurrently in the PE array.
        pv = psv.tile([64, 512], f32)
        for oc in range(H // 128):
            for half in range(2):
                nc.tensor.matmul(
                    pv[32 * half:32 * half + BPC, :],
                    lhsT=hid_sb[:, oc, :],
                    rhs=w_sb[:, oc, half * 512:(half + 1) * 512],
                    start=(oc == 0),
                    stop=(oc == H // 128 - 1),
                    tile_position=(0, 32 * half),
                )
        # one copy on ScalarE, one on VectorE -- they run in parallel,
        # halving this step of the v -> broadcast critical path
        for half, eng in ((0, nc.scalar.copy), (1, nc.vector.tensor_copy)):
            eng(
                v_pad[32 * half:32 * half + BPC, half * 512:(half + 1) * 512],
                pv[32 * half:32 * half + BPC, :],
            )

        # broadcast each v row across the 128 partitions; one tile per b so
        # the b=0 stream ops can start before later broadcasts finish.
        vbs = []
        for bi in range(BPC):
            vb_b = vbpool.tile([128, H], f32, tag=f"vb{bi}")
            for half in range(2):
                pb = psb.tile([128, 512], f32)
                sel = ind2 if half == 0 else ind3
                nc.tensor.matmul(
                    pb[:],
                    lhsT=sel[:, bi * 128:(bi + 1) * 128],
                    rhs=v_pad[:, half * 512:(half + 1) * 512],
                    start=True,
                    stop=True,
                )
                nc.scalar.copy(vb_b[:, half * 512:(half + 1) * 512], pb[:])
            vbs.append(vb_b)

        # main stream: E_t[tw, b] = sum_h enc[t, b, h] * v[b, h]; one E tile
        # per t-chunk so the inline PE transpose of chunk tc never blocks
        # the next chunk's accumulator writes. Per-chunk running max lands
        # in mcol so the final softmax only reduces [BPC, NTCH].
        pe = pse.tile([BPC, T], f32)
        pmax = smpool.tile([BPC, 1], f32)
        for tci in range(NTCH):
            # half tiles (b 0..3, 4..7) with separate DMAs so the first 4
            # STT ops only depend on the first half's arrival; the LAST
            # chunk uses quarter tiles (b pairs) to shorten the tail.
            if tci < NTCH - 1:
                parts = [encpool.tile([128, BPC // 2, H], f32, tag=f"eq{q}",
                                      name=f"eq{q}_{tci}") for q in range(2)]
                per = BPC // 2
            else:
                parts = [(encpool if q < 2 else encq).tile(
                    [128, BPC // 4, H], f32, tag=f"eq{q}",
                    name=f"eq{q}_{tci}") for q in range(4)]
                per = BPC // 4
            for q, pt in enumerate(parts):
                nc.sync.dma_start(
                    pt[:], enc[tci * TCH:(tci + 1) * TCH, q * per:(q + 1) * per, :])
            E_t = epool.tile([128, BPC], f32, tag=f"E{tci}")
            for bi in range(BPC):
                scr = scrpool.tile([128, H], f32)
                # out = (in0 * 1.0) * in1; accum_out = sum over free dim
                nc.vector.scalar_tensor_tensor(
                    out=scr[:],
                    in0=parts[bi // per][:, bi % per, :],
                    scalar=1.0,
                    in1=vbs[bi][:],
                    op0=Alu.mult,
                    op1=Alu.mult,
                    accum_out=E_t[:, bi:bi + 1],
                )
            nc.tensor.transpose(
                pe[:, tci * TCH:(tci + 1) * TCH],
                E_t[:],
                ident[:],
            )
            if tci == NTCH - 2:
                # running max over chunks 0..6 in the stream's DMA slack,
                # so the finale only reduces the final 128-col block
                nc.vector.tensor_reduce(
                    out=pmax[:], in_=pe[:, 0:(NTCH - 1) * TCH],
                    axis=mybir.AxisListType.X, op=Alu.max,
                )

        # softmax along free dim (t); energies read straight from PSUM.
        # Global max = max(partial over chunks 0..6, last block's max).
        bmax = smpool.tile([BPC, 1], f32)
        nc.vector.tensor_reduce(out=bmax[:], in_=pe[:, (NTCH - 1) * TCH:T],
                                axis=mybir.AxisListType.X, op=Alu.max)
        mx = smpool.tile([BPC, 1], f32)
        nc.vector.tensor_tensor(out=mx[:], in0=pmax[:], in1=bmax[:], op=Alu.max)
        nmx = smpool.tile([BPC, 1], f32)
        nc.vector.tensor_scalar_mul(nmx[:], mx[:], -1.0)
        ex = smpool.tile([BPC, T], f32)
        s = smpool.tile([BPC, 1], f32)
        nc.scalar.activation(
            ex[:], pe[:], mybir.ActivationFunctionType.Exp,
            bias=nmx[:], scale=1.0, accum_out=s[:],
        )
        r = smpool.tile([BPC, 1], f32)
        nc.vector.reciprocal(r[:], s[:])
        o = smpool.tile([BPC, T], f32)
        nc.vector.tensor_scalar_mul(o[:], ex[:], r[:])

        # Teardown trim: no SWDGE DMAs are used anywhere in this kernel, so
        # the per-range gpsimd dma_reset in the tail's semaphore cleanup is
        # dead weight (~1-3us). sem_clear still runs.
        nc.gpsimd.dma_reset = lambda *a, **k: None
        nc.sync.dma_start(out[:], o[:])

    _split_multi_waits(nc)
    return nc


def _get_bass():
    if "nc" not in _BASS_CACHE:
        _BASS_CACHE["nc"] = _build_bass()
    return _BASS_CACHE["nc"]


def make_in_maps(hidden, encoder_outputs, W, b):
    """Shard full inputs into per-core input maps (host-side layout prep)."""
    hidden = np.asarray(hidden, dtype=np.float32)
    encoder_outputs = np.asarray(encoder_outputs, dtype=np.float32)
    W = np.asarray(W, dtype=np.float32)
    hidT = np.ascontiguousarray(hidden[0].T)  # [H, B]
    in_maps = []
    for i in range(NCORES):
        # [H, BPC] -> [oc, 128, BPC] -> [128, oc, BPC] -> [128, oc*BPC]
        hid_slice = hidT[:, i * BPC:(i + 1) * BPC]
        hid_prep = np.ascontiguousarray(
            hid_slice.reshape(H // 128, 128, BPC).transpose(1, 0, 2).reshape(128, -1)
        )
        in_maps.append({
            "enc": np.ascontiguousarray(encoder_outputs[:, i * BPC:(i + 1) * BPC, :]),
            "hidt": hid_prep,
            "w": W,
        })
    return in_maps


def run_on_hw(in_maps, trace=False):
    from concourse.bass_utils import run_bass_kernel_spmd

    nc = _get_bass()
    return run_bass_kernel_spmd(nc, in_maps, list(range(NCORES)), trace=trace)


def kernel(hidden, encoder_outputs, W, b):
    in_maps = make_in_maps(hidden, encoder_outputs, W, b)
    res = run_on_hw(in_maps, trace=False)
    parts = [np.asarray(res.results[i]["out"]) for i in range(NCORES)]
    energies_sm = np.concatenate(parts, axis=0)  # [B, T]
    return energies_sm[:, None, :].astype(np.float32)



# revision 2
# speedup vs baseline: 1.5050x; 1.5050x over previous
"""Trainium2 Bass kernel for additive-attention energies + softmax.

Computes, for hidden [1, B, H], encoder_outputs [T, B, H], W [H, H], b [H]:
    proj[t,b,o]  = sum_h enc[t,b,h] * W[o,h] + b[o]
    energies[b,t] = sum_o hidden[0,b,o] * proj[t,b,o]
    out = softmax(energies, axis=-1)[:, None, :]            # [B, 1, T]

Algebraic rewrite used on-device:
    energies[b,t] = (hidden[b] @ W) . enc[t,b]  +  hidden[b] . b
The second term is constant in t, so it drops out of the softmax entirely.

v2 design (PE-centric, fp16 stream):
  * The host casts enc/W/hidden to fp16 and pre-transposes enc per-core to
    [b][h][t].  This halves HBM traffic (the binding constraint) AND puts h
    on SBUF partitions so the dot products run on the tensor engine:
       E[b, t-half] += matmul(lhsT=vT[:, hc, b:b+1], rhs=enc[b, hc, :, t-half])
    accumulated over the 8 h-chunks in PSUM (fp32).
  * vT[h, b] = sum_o W[o, h] hid[o, b] is computed directly transposed with
    lhsT=W chunks — no broadcast/indicator matmuls and no PE transposes.
  * tile_position=(0, 32*(b//2)) parks each b's [1, T] energy row on PSUM
    partition 32*(b//2), bank pair b%2 — the [8, T] energies need no
    transpose; softmax (max/exp/scale) runs per-b in DMA slack on the
    otherwise idle vector+scalar engines.  Consecutive b's in compute order
    [0,3,6,1,4,7,2,5] alternate both column-strips (LDW pull-ahead) and
    PSUM bank pairs (no PE-write/DVE-read same-bank collisions).
  * fp16 rounding of enc/W/hid/vT gives rel err ~5.5e-3 (tolerance 2e-2).

Sharding: data-parallel over batch. Core i handles batches [8i, 8i+8):
  enc slice [8, H, T] fp16 (16 MB), W fp16 replicated (2 MB).
Per-core output is [8, T] fp32; host concatenates to [B, 1, T].
Per-core roofline: ~18 MB of HBM reads at ~358-368 GB/s ~= 50 us.
"""

import sys

import numpy as np

for _p in ("/opt/trn_rl_repo",):
    if _p not in sys.path:
        sys.path.insert(0, _p)

T, B, H = 1024, 64, 1024
NCORES = 8
BPC = B // NCORES  # batches per core
NHC = H // 128     # h-chunks
ENC_BUFS = 6

_BASS_CACHE = {}


def _split_multi_waits(nc):
    """This walrus build rejects >1 semaphore wait per instruction for
    several instruction types (Drain/CTRL, LDWEIGHTS, ...). Normalize every
    instruction to <=1 wait: hoist extra waits onto fresh single-wait drain
    clones inserted immediately before it on the same engine (engines are
    serial, so semantics are identical)."""
    import copy

    template = None
    for fn in nc.m.functions:
        for bb in fn.blocks:
            for inst in bb.instructions:
                if type(inst).__name__ == "InstDrain":
                    template = inst
                    break
            if template is not None:
                break
        if template is not None:
            break
    assert template is not None, "no InstDrain found to use as wait-carrier"

    uid = [0]
    for fn in nc.m.functions:
        for bb in fn.blocks:
            out = []
            changed = False
            for inst in bb.instructions:
                si = inst.sync_info
                if si is not None and si.on_wait and len(si.on_wait) > 1:
                    waits = list(si.on_wait)
                    for w in waits[:-1]:
                        d = copy.deepcopy(template)
                        d.name = f"waitsplit-{uid[0]}"
                        uid[0] += 1
                        d.engine = inst.engine
                        dsi = d.sync_info
                        dsi.on_wait = [w]
                        if dsi.on_update:
                            dsi.on_update = []
                        out.append(d)
                        nc.register_instruction(d, overwrite=True)
                    si.on_wait = [waits[-1]]
                    changed = True
                out.append(inst)
            if changed:
                try:
                    bb.instructions = out
                except Exception:
                    bb.instructions.clear()
                    bb.instructions.extend(out)


def _build_bass():
    """Build the per-core Bass program (same program on all 8 cores)."""
    from contextlib import ExitStack

    import concourse.bass as bass
    import concourse.mybir as mybir
    import concourse.tile as tile

    f32 = mybir.dt.float32
    f16 = mybir.dt.float16
    Alu = mybir.AluOpType
    AxX = mybir.AxisListType.X

    nc = bass.Bass("TRN2")
    enc_h = nc.dram_tensor("enc", [BPC, H, T], f16, kind="ExternalInput")
    # hidt arrives host-prearranged as [128, H/128 * BPC]: partition p of
    # oc-chunk holds hidden[o = oc*128 + p, b] — the exact rhs layout.
    hid_h = nc.dram_tensor("hidt", [128, NHC * BPC], f16, kind="ExternalInput")
    w_h = nc.dram_tensor("w", [H, H], f16, kind="ExternalInput")
    out_h = nc.dram_tensor("out", [BPC, T], f32, kind="ExternalOutput")

    enc, hidT, w, out = enc_h.ap(), hid_h.ap(), w_h.ap(), out_h.ap()

    with tile.TileContext(nc) as tc, ExitStack() as ctx:
        const = ctx.enter_context(tc.tile_pool(name="const", bufs=1))
        wpool = ctx.enter_context(tc.tile_pool(name="wpool", bufs=1))
        hpool = ctx.enter_context(tc.tile_pool(name="hpool", bufs=1))
        vtpool = ctx.enter_context(tc.tile_pool(name="vtp", bufs=1))
        encpool = ctx.enter_context(tc.tile_pool(name="encp", bufs=ENC_BUFS))
        smpool = ctx.enter_context(tc.tile_pool(name="sm", bufs=1))
        opool = ctx.enter_context(tc.tile_pool(name="op", bufs=1))
        psE = ctx.enter_context(tc.tile_pool(name="psE", bufs=1, space="PSUM"))
        psvt = ctx.enter_context(tc.tile_pool(name="psvt", bufs=1, space="PSUM"))
        psw = ctx.enter_context(tc.tile_pool(name="psw", bufs=1, space="PSUM"))

        # Preload the ScalarE activation table (Copy lives in the same set
        # as Exp) during the preamble -- otherwise the first ACT op on the
        # critical path eats a ~2.7us ACT_TABLE_LOAD.
        actwarm = const.tile([1, 1], f32)
        nc.scalar.activation(actwarm[:], actwarm[:],
                             mybir.ActivationFunctionType.Exp)

        # PE warm-up: junk matmuls so the HAM un-throttles the PE clock
        # (1.2 -> 2.4 GHz) before the vT matmul chain.
        junk = const.tile([128, 512], f16)
        nc.gpsimd.memset(junk[:], 0.0)
        for wi in range(8):
            pw = psw.tile([128, 512], f32, tag="warm")
            nc.tensor.matmul(pw[:], lhsT=junk[:, 0:128], rhs=junk[:],
                             start=True, stop=True)

        # hid (prearranged) -> SBUF [128, oc, b]
        hid_sb = hpool.tile([128, NHC, BPC], f16)
        nc.sync.dma_start(hid_sb[:], hidT.rearrange("p (oc b) -> p oc b", b=BPC))

        # W [o, h] -> SBUF [128, oc, h] in two h-halves so the first vT
        # accumulation groups can start at half arrival.
        w_sb = wpool.tile([128, NHC, H], f16)
        w_r = w.rearrange("(oc p) h -> p oc h", p=128)
        for hh in range(2):
            nc.sync.dma_start(w_sb[:, :, hh * 512:(hh + 1) * 512],
                              w_r[:, :, hh * 512:(hh + 1) * 512])

        # vT[h, b] = sum_o W[o, h] hid[o, b], computed directly transposed:
        # per h-chunk hc, accumulate over o-chunks with lhsT = W block.
        # Groups are strictly sequential per PSUM bank, so each group's
        # start=True bank-bit clear cannot disturb a live accumulation.
        vt_ps = psvt.tile([128, NHC, BPC], f32)
        for hc in range(NHC):
            for oc in range(NHC):
                nc.tensor.matmul(
                    vt_ps[:, hc, :],
                    lhsT=w_sb[:, oc, hc * 128:(hc + 1) * 128],
                    rhs=hid_sb[:, oc, :],
                    start=(oc == 0),
                    stop=(oc == NHC - 1),
                )
        vt_sb = vtpool.tile([128, NHC, BPC], f16)
        nc.scalar.copy(vt_sb[:], vt_ps[:])

        # Main stream.  E[g, col*1024 + half*512 + t] for g = 32*(b//2),
        # col = b%2: each b owns one PSUM partition row and one bank pair.
        E = psE.tile([128, 2 * T], f32)
        mx = smpool.tile([128, 2], f32)
        nmx = smpool.tile([128, 2], f32)
        s = smpool.tile([128, 2], f32)
        r = smpool.tile([128, 2], f32)
        ex = smpool.tile([128, 2 * T], f32)
        o_sb = opool.tile([128, 2 * T], f32)

        b_order = [0, 3, 6, 1, 4, 7, 2, 5]
        for b in b_order:
            g = 32 * (b // 2)
            col = b % 2
            # two DMA half-tiles (h-chunk groups 0-3 / 4-7) so the first 8
            # matmuls only depend on the first half's arrival
            parts = []
            for hh in range(2):
                et = encpool.tile([128, 4, T], f16, tag=f"eq{hh}",
                                  name=f"eq{hh}_{b}")
                nc.sync.dma_start(
                    et[:],
                    enc[b, hh * 512:(hh + 1) * 512, :].rearrange(
                        "(hc p) t -> p hc t", p=128),
                )
                parts.append(et)
            for hc in range(NHC):
                for half in range(2):
                    off = col * T + half * 512
                    nc.tensor.matmul(
                        E[g:g + 1, off:off + 512],
                        lhsT=vt_sb[:, hc, b:b + 1],
                        rhs=parts[hc // 4][:, hc % 4, half * 512:(half + 1) * 512],
                        start=(hc == 0),
                        stop=(hc == NHC - 1),
                        tile_position=(0, g),
                    )
            # per-b softmax in stream slack: max -> exp(+accum) -> 1/s -> mul
            nc.vector.tensor_reduce(
                out=mx[g:g + 1, col:col + 1],
                in_=E[g:g + 1, col * T:(col + 1) * T],
                axis=AxX, op=Alu.max,
            )
            nc.vector.tensor_scalar_mul(
                nmx[g:g + 1, col:col + 1], mx[g:g + 1, col:col + 1], -1.0)
            nc.scalar.activation(
                ex[g:g + 1, col * T:(col + 1) * T],
                E[g:g + 1, col * T:(col + 1) * T],
                mybir.ActivationFunctionType.Exp,
                bias=nmx[g:g + 1, col:col + 1], scale=1.0,
                accum_out=s[g:g + 1, col:col + 1],
            )
            nc.vector.reciprocal(r[g:g + 1, col:col + 1], s[g:g + 1, col:col + 1])
            nc.vector.tensor_scalar_mul(
                o_sb[g:g + 1, col * T:(col + 1) * T],
                ex[g:g + 1, col * T:(col + 1) * T],
                r[g:g + 1, col:col + 1],
            )
            nc.sync.dma_start(out[b:b + 1, :],
                              o_sb[g:g + 1, col * T:(col + 1) * T])

        # Teardown trim: no SWDGE DMAs are used anywhere in this kernel, so
        # the per-range gpsimd dma_reset in the tail's semaphore cleanup is
        # dead weight (~1-3us). sem_clear still runs.
        nc.gpsimd.dma_reset = lambda *a, **k: None

    _split_multi_waits(nc)
    return nc


def _get_bass():
    if "nc" not in _BASS_CACHE:
        _BASS_CACHE["nc"] = _build_bass()
    return _BASS_CACHE["nc"]


def make_in_maps(hidden, encoder_outputs, W, b):
    """Shard full inputs into per-core input maps (host-side layout prep)."""
    hidden = np.asarray(hidden, dtype=np.float32)
    enc = np.asarray(encoder_outputs, dtype=np.float32)
    W16 = np.asarray(W, dtype=np.float32).astype(np.float16)
    hid16 = hidden[0].astype(np.float16)          # [B, o]
    hidT = np.ascontiguousarray(hid16.T)          # [o, B]
    enc16 = enc.astype(np.float16)                # [T, B, H]
    in_maps = []
    for i in range(NCORES):
        # [T, 8, H] -> [8, H, T] (b-major, h on partitions, t contiguous)
        enc_t = np.ascontiguousarray(
            enc16[:, i * BPC:(i + 1) * BPC, :].transpose(1, 2, 0))
        # [o, BPC] -> [oc, 128, BPC] -> [128, oc, BPC] -> [128, oc*BPC]
        hid_slice = hidT[:, i * BPC:(i + 1) * BPC]
        hid_prep = np.ascontiguousarray(
            hid_slice.reshape(H // 128, 128, BPC).transpose(1, 0, 2).reshape(128, -1)
        )
        in_maps.append({
            "enc": enc_t,
            "hidt": hid_prep,
            "w": W16,
        })
    return in_maps


def run_on_hw(in_maps, trace=False):
    from concourse.bass_utils import run_bass_kernel_spmd

    nc = _get_bass()
    return run_bass_kernel_spmd(nc, in_maps, list(range(NCORES)), trace=trace)


def kernel(hidden, encoder_outputs, W, b):
    in_maps = make_in_maps(hidden, encoder_outputs, W, b)
    res = run_on_hw(in_maps, trace=False)
    parts = [np.asarray(res.results[i]["out"]) for i in range(NCORES)]
    energies_sm = np.concatenate(parts, axis=0)  # [B, T]
    return energies_sm[:, None, :].astype(np.float32)


# revision 5
# speedup vs baseline: 1.7121x; 1.1376x over previous
"""Trainium2 Bass kernel for additive-attention energies + softmax.

Computes, for hidden [1, B, H], encoder_outputs [T, B, H], W [H, H], b [H]:
    proj[t,b,o]  = sum_h enc[t,b,h] * W[o,h] + b[o]
    energies[b,t] = sum_o hidden[0,b,o] * proj[t,b,o]
    out = softmax(energies, axis=-1)[:, None, :]            # [B, 1, T]

Algebraic rewrite used on-device:
    energies[b,t] = (hidden[b] @ W) . enc[t,b]  +  hidden[b] . b
The second term is constant in t, so it drops out of the softmax entirely.

v3 design (PE-centric, fp16 stream):
  * Host casts enc/W/hidden to fp16 and pre-transposes enc per-core to
    [b][h][t].  Halves HBM traffic (the binding constraint) AND puts h on
    SBUF partitions so the dot products run on the tensor engine:
       E[b, t-half] += matmul(lhsT=vT[:, hc, b:b+1], rhs=enc[b, hc, :, th])
    accumulated over the 8 h-chunks in PSUM (fp32).
  * vT[h, b] = sum_o W[o, h] hid[o, b] is computed directly transposed with
    lhsT=W chunks — no broadcast matmuls, no PE transposes.
  * b's are processed as interleaved PAIRS from different 32-column strips
    (tile_position=(0, 32*(b//2))) so each LDWEIGHTS streams into idle
    sub-arrays while the other b's matmuls run; MMs pipeline at ~N cycles.
  * PSUM layout: E is the whole 8-bank space [128, 4096]; consecutive
    pairs alternate bank quads so softmax reads never touch banks the PE
    is writing.  vT accumulation and PE warm-up reuse E banks (strictly
    before their first stream use).
  * Per-b softmax runs in stream slack: one ACT copy PSUM->SBUF, then
    max/exp/scale on SBUF.  Output DMAs issue from the scalar queue so the
    sync queue stays a pure prefetch FIFO (out-DMAs on the sync queue
    throttled the enc prefetch to the softmax cadence in v2: 85 us).
  * fp16 rounding gives rel err ~6e-3 (tolerance 2e-2).

Sharding: data-parallel over batch. Core i handles batches [8i, 8i+8):
  enc slice [8, H, T] fp16 (16 MB), W fp16 replicated (2 MB).
Per-core output is [8, T] fp32; host concatenates to [B, 1, T].
Per-core roofline: ~19 MB of HBM reads at ~358-368 GB/s ~= 52 us.
"""

import sys

import numpy as np

for _p in ("/opt/trn_rl_repo",):
    if _p not in sys.path:
        sys.path.insert(0, _p)

T, B, H = 1024, 64, 1024
NCORES = 8
BPC = B // NCORES  # batches per core
NHC = H // 128     # h-chunks
ENC_BUFS = 8

# b-pairs: within a pair the two b's sit on different column strips (LDW
# overlap) and different bank pairs; consecutive pairs alternate bank quads.
B_PAIRS = [(0, 3), (6, 1), (4, 7), (2, 5)]
# PSUM free-offset (fp32 elems) of each b's [1, 2*512] energy row.
E_OFF = {0: 0, 3: 1024, 6: 2048, 1: 3072, 4: 0, 7: 1024, 2: 2048, 5: 3072}

_BASS_CACHE = {}


def _split_multi_waits(nc):
    """This walrus build rejects >1 semaphore wait per instruction for
    several instruction types (Drain/CTRL, LDWEIGHTS, ...). Normalize every
    instruction to <=1 wait: hoist extra waits onto fresh single-wait drain
    clones inserted immediately before it on the same engine (engines are
    serial, so semantics are identical)."""
    import copy

    template = None
    for fn in nc.m.functions:
        for bb in fn.blocks:
            for inst in bb.instructions:
                if type(inst).__name__ == "InstDrain":
                    template = inst
                    break
            if template is not None:
                break
        if template is not None:
            break
    assert template is not None, "no InstDrain found to use as wait-carrier"

    uid = [0]
    for fn in nc.m.functions:
        for bb in fn.blocks:
            out = []
            changed = False
            for inst in bb.instructions:
                si = inst.sync_info
                if si is not None and si.on_wait and len(si.on_wait) > 1:
                    waits = list(si.on_wait)
                    for w in waits[:-1]:
                        d = copy.deepcopy(template)
                        d.name = f"waitsplit-{uid[0]}"
                        uid[0] += 1
                        d.engine = inst.engine
                        dsi = d.sync_info
                        dsi.on_wait = [w]
                        if dsi.on_update:
                            dsi.on_update = []
                        out.append(d)
                        nc.register_instruction(d, overwrite=True)
                    si.on_wait = [waits[-1]]
                    changed = True
                out.append(inst)
            if changed:
                try:
                    bb.instructions = out
                except Exception:
                    bb.instructions.clear()
                    bb.instructions.extend(out)


def _build_bass():
    """Build the per-core Bass program (same program on all 8 cores)."""
    from contextlib import ExitStack

    import concourse.bass as bass
    import concourse.mybir as mybir
    import concourse.tile as tile

    f32 = mybir.dt.float32
    f16 = mybir.dt.float16
    Alu = mybir.AluOpType
    AxX = mybir.AxisListType.X

    nc = bass.Bass("TRN2")
    enc_h = nc.dram_tensor("enc", [BPC, H, T], f16, kind="ExternalInput")
    # hidt arrives host-prearranged as [128, H/128 * BPC]: partition p of
    # oc-chunk holds hidden[o = oc*128 + p, b] — the exact rhs layout.
    hid_h = nc.dram_tensor("hidt", [128, NHC * BPC], f16, kind="ExternalInput")
    w_h = nc.dram_tensor("w", [H, H], f16, kind="ExternalInput")
    out_h = nc.dram_tensor("out", [BPC, T], f32, kind="ExternalOutput")

    enc, hidT, w, out = enc_h.ap(), hid_h.ap(), w_h.ap(), out_h.ap()

    with tile.TileContext(nc) as tc, ExitStack() as ctx:
        const = ctx.enter_context(tc.tile_pool(name="const", bufs=1))
        wpool = ctx.enter_context(tc.tile_pool(name="wpool", bufs=1))
        hpool = ctx.enter_context(tc.tile_pool(name="hpool", bufs=1))
        vtpool = ctx.enter_context(tc.tile_pool(name="vtp", bufs=1))
        encpool = ctx.enter_context(tc.tile_pool(name="encp", bufs=ENC_BUFS))
        smpool = ctx.enter_context(tc.tile_pool(name="sm", bufs=1))
        psE = ctx.enter_context(tc.tile_pool(name="psE", bufs=1, space="PSUM"))

        # Preload the ScalarE activation table (Copy lives in the same set
        # as Exp) during the preamble -- otherwise the first ACT op on the
        # critical path eats a ~2.7us ACT_TABLE_LOAD.
        actwarm = const.tile([1, 1], f32)
        nc.scalar.activation(actwarm[:], actwarm[:],
                             mybir.ActivationFunctionType.Exp)

        # E is the whole PSUM: 8 banks. vT accumulation and warm-up junk
        # reuse E banks strictly before their first stream use.
        E = psE.tile([128, 4096], f32)

        # PE warm-up: junk matmuls so the HAM un-throttles the PE clock
        # (1.2 -> 2.4 GHz) before the vT matmul chain. Lands in bank 7.
        junk = const.tile([128, 512], f16)
        nc.gpsimd.memset(junk[:], 0.0)
        for wi in range(8):
            nc.tensor.matmul(E[:, 3584:4096], lhsT=junk[:, 0:128], rhs=junk[:],
                             start=True, stop=True)

        # hid (prearranged) -> SBUF [128, oc, b]
        hid_sb = hpool.tile([128, NHC, BPC], f16)
        nc.sync.dma_start(hid_sb[:], hidT.rearrange("p (oc b) -> p oc b", b=BPC))

        # W [o, h] -> SBUF [128, oc, h] in two h-halves so the first vT
        # accumulation groups can start at half arrival.
        w_sb = wpool.tile([128, NHC, H], f16)
        w_r = w.rearrange("(oc p) h -> p oc h", p=128)
        for hh in range(2):
            nc.sync.dma_start(w_sb[:, :, hh * 512:(hh + 1) * 512],
                              w_r[:, :, hh * 512:(hh + 1) * 512])

        # vT[h, b] = sum_o W[o, h] hid[o, b], computed directly transposed:
        # per h-chunk hc, accumulate over o-chunks with lhsT = W block.
        # Lives in E bank 0 (b=0's stream group reuses it later); flat
        # layout [128, hc*BPC + b].
        for hc in range(NHC):
            for oc in range(NHC):
                nc.tensor.matmul(
                    E[:, hc * BPC:(hc + 1) * BPC],
                    lhsT=w_sb[:, oc, hc * 128:(hc + 1) * 128],
                    rhs=hid_sb[:, oc, :],
                    start=(oc == 0),
                    stop=(oc == NHC - 1),
                )
        vt_sb = vtpool.tile([128, NHC * BPC], f16)
        nc.scalar.copy(vt_sb[:], E[:, 0:NHC * BPC])

        # SBUF-side softmax staging: per b, row g = 32*(b//2), col b%2.
        e_sb = smpool.tile([128, 2 * T], f32)
        mx = smpool.tile([128, 2], f32)
        nmx = smpool.tile([128, 2], f32)
        s = smpool.tile([128, 2], f32)
        r = smpool.tile([128, 2], f32)
        o_sb = smpool.tile([128, 2 * T], f32)

        def softmax_b(b):
            g = 32 * (b // 2)
            col = b % 2
            eoff = E_OFF[b]
            # one narrow PSUM read, then everything on SBUF
            nc.scalar.copy(e_sb[g:g + 1, col * T:(col + 1) * T],
                           E[g:g + 1, eoff:eoff + T])
            nc.vector.tensor_reduce(
                out=mx[g:g + 1, col:col + 1],
                in_=e_sb[g:g + 1, col * T:(col + 1) * T],
                axis=AxX, op=Alu.max,
            )
            nc.vector.tensor_scalar_mul(
                nmx[g:g + 1, col:col + 1], mx[g:g + 1, col:col + 1], -1.0)
            nc.scalar.activation(
                o_sb[g:g + 1, col * T:(col + 1) * T],
                e_sb[g:g + 1, col * T:(col + 1) * T],
                mybir.ActivationFunctionType.Exp,
                bias=nmx[g:g + 1, col:col + 1], scale=1.0,
                accum_out=s[g:g + 1, col:col + 1],
            )
            nc.vector.reciprocal(r[g:g + 1, col:col + 1],
                                 s[g:g + 1, col:col + 1])
            nc.vector.tensor_scalar_mul(
                o_sb[g:g + 1, col * T:(col + 1) * T],
                o_sb[g:g + 1, col * T:(col + 1) * T],
                r[g:g + 1, col:col + 1],
            )
            # out-DMA from the scalar queue: the sync queue must stay a
            # pure prefetch FIFO or this issue would gate later enc DMAs.
            nc.scalar.dma_start(out[b:b + 1, :],
                                o_sb[g:g + 1, col * T:(col + 1) * T])

        # Main stream, two b's interleaved per pair.
        for bA, bB in B_PAIRS:
            tiles = {}
            for b in (bA, bB):
                parts = []
                for hh in range(2):
                    et = encpool.tile([128, 4, T], f16, tag=f"eq{hh}",
                                      name=f"eq{hh}_{b}")
                    nc.sync.dma_start(
                        et[:],
                        enc[b, hh * 512:(hh + 1) * 512, :].rearrange(
                            "(hc p) t -> p hc t", p=128),
                    )
                    parts.append(et)
                tiles[b] = parts
            for hc in range(NHC):
                for b in (bA, bB):
                    g = 32 * (b // 2)
                    eoff = E_OFF[b]
                    for half in range(2):
                        off = eoff + half * 512
                        nc.tensor.matmul(
                            E[g:g + 1, off:off + 512],
                            lhsT=vt_sb[:, hc * BPC + b:hc * BPC + b + 1],
                            rhs=tiles[b][hc // 4][:, hc % 4,
                                                  half * 512:(half + 1) * 512],
                            start=(hc == 0),
                            stop=(hc == NHC - 1),
                            tile_position=(0, g),
                        )
            softmax_b(bA)
            softmax_b(bB)

        # Teardown trim: no SWDGE DMAs are used anywhere in this kernel, so
        # the per-range gpsimd dma_reset in the tail's semaphore cleanup is
        # dead weight (~1-3us). sem_clear still runs.
        nc.gpsimd.dma_reset = lambda *a, **k: None

    _split_multi_waits(nc)
    return nc


def _get_bass():
    if "nc" not in _BASS_CACHE:
        _BASS_CACHE["nc"] = _build_bass()
    return _BASS_CACHE["nc"]


def make_in_maps(hidden, encoder_outputs, W, b):
    """Shard full inputs into per-core input maps (host-side layout prep)."""
    hidden = np.asarray(hidden, dtype=np.float32)
    enc = np.asarray(encoder_outputs, dtype=np.float32)
    W16 = np.asarray(W, dtype=np.float32).astype(np.float16)
    hid16 = hidden[0].astype(np.float16)          # [B, o]
    hidT = np.ascontiguousarray(hid16.T)          # [o, B]
    enc16 = enc.astype(np.float16)                # [T, B, H]
    in_maps = []
    for i in range(NCORES):
        # [T, 8, H] -> [8, H, T] (b-major, h on partitions, t contiguous)
        enc_t = np.ascontiguousarray(
            enc16[:, i * BPC:(i + 1) * BPC, :].transpose(1, 2, 0))
        # [o, BPC] -> [oc, 128, BPC] -> [128, oc, BPC] -> [128, oc*BPC]
        hid_slice = hidT[:, i * BPC:(i + 1) * BPC]
        hid_prep = np.ascontiguousarray(
            hid_slice.reshape(H // 128, 128, BPC).transpose(1, 0, 2).reshape(128, -1)
        )
        in_maps.append({
            "enc": enc_t,
            "hidt": hid_prep,
            "w": W16,
        })
    return in_maps


def run_on_hw(in_maps, trace=False):
    from concourse.bass_utils import run_bass_kernel_spmd

    nc = _get_bass()
    return run_bass_kernel_spmd(nc, in_maps, list(range(NCORES)), trace=trace)


def kernel(hidden, encoder_outputs, W, b):
    in_maps = make_in_maps(hidden, encoder_outputs, W, b)
    res = run_on_hw(in_maps, trace=False)
    parts = [np.asarray(res.results[i]["out"]) for i in range(NCORES)]
    energies_sm = np.concatenate(parts, axis=0)  # [B, T]
    return energies_sm[:, None, :].astype(np.float32)
